# revision 58
# baseline (speedup 1.0000x reference)
# Multi-head attention (B=2, S=2048, D=1024, H=16) on 8 TRN2 NeuronCores.
#
# Sharding: core c handles batch b = c//4 and head-group hg = c%4 (4 heads,
# channel slice J = hg*256 : (hg+1)*256).  Each core computes
#   Q^T/K^T = W^T x^T (bias via DVE tensor_scalar_add), V = x W (+bias),
#   S^T_h = K_h^T^T-contraction (d on partitions)  -> exp on ScalarE -> fp8,
#   O^T_h = [V|1]^T P^T_h  via fp8 DoubleRow matmuls (K=256 per matmul),
#   y_partial = O^T^T Wo_slice  (bf16).
# Host sums the 4 partials per batch and adds bo.
#
# Schedule: ScalarE (exp, ~150us of ACTIVATE) is the conveyor.  The kernel
# starts the first scores unit as early as possible (chunk-major xT DMA +
# minimal pair-0 prologue), keeps one exp tile always queued, and paces all
# other PE work (projections, attn@V, y-proj) into the PE slack between
# score matmuls.  attn@V is consumed eagerly (~4 kb behind exp) so the tail
# after the last exp is short.
#
# Precision: scores matmuls bf16; P (exp output) and V in fp8e4m3 for the
# DoubleRow attn@V.  Only ScalarE can convert to fp8 on-chip (DVE/GPSIMD
# silently write zeros), so V goes psum->fp8 through ACT Copy (block-pair
# groups to amortize the ~352cyc/instr overhead).  The denominator ones
# column rides in V; its reciprocal is computed on DVE with the approx
# fast-recip (base partition 0 only!) and broadcast via one DRAM bounce.

import numpy as np

B = 2
S = 2048
D = 1024
H = 16
DH = 64
NCORES = 8
HL = 4            # heads per core
J = HL * DH       # 256: per-core channel slice of D
PAIRS = 2         # head-pairs per core

_cache = {}
DEBUG_DUMP = False


def _build_module(seq=S):
    import concourse.bass as bass
    import concourse.mybir as mybir
    import concourse.tile as tile

    from concourse import bacc

    dt = mybir.dt
    f32 = dt.float32
    bf16 = dt.bfloat16
    f8 = dt.float8e4
    AF = mybir.ActivationFunctionType
    DR = mybir.MatmulPerfMode.DoubleRow

    KB = seq // 128          # key blocks (128-partition tiles of the key dim)
    KBP = KB // 2            # key block pairs (DoubleRow: K=256 per matmul)
    QC = min(512, seq)       # query chunk (matmul free dim)
    NQ = seq // QC           # query chunks
    NCH = min(512, seq)      # projection free-dim chunk
    NP = seq // NCH          # projection chunks
    KT = D // 128            # contraction tiles for projections (8)

    nc = bacc.Bacc("TRN2", target_bir_lowering=False, debug=False)

    xT_d = nc.dram_tensor("xT", [D, seq], bf16, kind="ExternalInput").ap()
    # QKV weights in two big-row blobs: row p holds [k, which, j] ->
    # W_which[k*128+p, js[j]]  (wqk: which in {q,k}; wv separate so the
    # critical first DMA is smaller)
    wqk_d = nc.dram_tensor("wqk", [128, KT * 2 * J], bf16,
                           kind="ExternalInput").ap()
    wv2_d = nc.dram_tensor("wv2", [128, KT * J], bf16,
                           kind="ExternalInput").ap()
    wo_d = nc.dram_tensor("wo", [J, D], bf16, kind="ExternalInput").ap()
    bqk_d = nc.dram_tensor("bqk", [128, 4], f32, kind="ExternalInput").ap()
    bv_d = nc.dram_tensor("bv", [1, 2 * J], bf16, kind="ExternalInput").ap()
    y_d = nc.dram_tensor("y", [seq, D], bf16, kind="ExternalOutput").ap()
    if DEBUG_DUMP:
        dbg_qt = nc.dram_tensor("dbg_qt", [128, seq], bf16,
                                kind="ExternalOutput").ap()
        dbg_kt = nc.dram_tensor("dbg_kt", [128, seq], bf16,
                                kind="ExternalOutput").ap()
        dbg_v = nc.dram_tensor("dbg_v", [128, (seq // 256) * 2 * HL * 68], f8,
                               kind="ExternalOutput").ap()
        dbg_pt = nc.dram_tensor("dbg_pt", [128, (seq // 256) * 2 * 2 * 512],
                                f8, kind="ExternalOutput").ap()
        dbg_dnm = nc.dram_tensor("dbg_dnm", [1, 512], f32,
                                 kind="ExternalOutput").ap()
        dbg_rr = nc.dram_tensor("dbg_rr", [1, 512], f32,
                                kind="ExternalOutput").ap()
        dbg_ot = nc.dram_tensor("dbg_ot", [128, seq], bf16,
                                kind="ExternalOutput").ap()

    with tile.TileContext(nc) as tc:
        import contextlib
        ctx = contextlib.ExitStack()
        with ctx:
            xt_pool = ctx.enter_context(tc.tile_pool(name="xt", bufs=1))
            w_pool = ctx.enter_context(tc.tile_pool(name="w", bufs=1))
            qk_pool = ctx.enter_context(tc.tile_pool(name="qk", bufs=1))
            v_pool = ctx.enter_context(tc.tile_pool(name="v", bufs=1))
            pt_pool = ctx.enter_context(tc.tile_pool(name="pt", bufs=2))
            ot_pool = ctx.enter_context(tc.tile_pool(name="ot", bufs=1))
            sm_pool = ctx.enter_context(tc.tile_pool(name="sm", bufs=2))
            yb_pool = ctx.enter_context(tc.tile_pool(name="yb", bufs=2))
            psS_pool = ctx.enter_context(
                tc.tile_pool(name="psS", bufs=2, space="PSUM"))
            psO_pool = ctx.enter_context(
                tc.tile_pool(name="psO", bufs=2, space="PSUM"))
            mm_pool = ctx.enter_context(
                tc.tile_pool(name="mm", bufs=2, space="PSUM"))
            dram_pool = ctx.enter_context(
                tc.tile_pool(name="dscr", bufs=2, space="DRAM"))

            # ---- persistent SBUF tensors ----
            xt_sb = [xt_pool.tile([128, seq], bf16, tag=f"xt{k}",
                                  name=f"xt{k}") for k in range(KT)]
            wqk_sb = w_pool.tile([128, KT * 2 * J], bf16, tag="wqk",
                                 name="wqk")
            wv2_sb = w_pool.tile([128, KT * J], bf16, tag="wv2", name="wv2")
            wq_sb = [wqk_sb[:, (2 * k + 0) * J:(2 * k + 1) * J]
                     for k in range(KT)]
            wk_sb = [wqk_sb[:, (2 * k + 1) * J:(2 * k + 2) * J]
                     for k in range(KT)]
            wv_sb = [wv2_sb[:, k * J:(k + 1) * J] for k in range(KT)]
            wo_sb = [w_pool.tile([128, D], bf16, tag=f"wo{p}",
                                 name=f"wo{p}") for p in range(PAIRS)]
            bqk_sb = w_pool.tile([128, 4], f32, tag="bqk", name="bqk")
            bv_sb = w_pool.tile([1, 2 * J], bf16, tag="bv", name="bv")
            ones_sb = w_pool.tile([1, 128], bf16, tag="ones", name="ones")

            qt_sb = [qk_pool.tile([128, seq], bf16, tag=f"qt{p}",
                                  name=f"qt{p}") for p in range(PAIRS)]
            kt_sb = [qk_pool.tile([128, seq], bf16, tag=f"kt{p}",
                                  name=f"kt{p}") for p in range(PAIRS)]
            # V in fp8, key-block pairs packed for DoubleRow:
            # [128, kbp, sub, head, 68] ; col 64 = 1.0 (fused denominator),
            # cols 65..67 pad so the DoubleRow sub-stride (272B) is 16B-mult.
            v_sb = v_pool.tile([128, KBP, 2, HL, 68], f8, tag="v", name="v")
            # key blocks 12..15 keep V (and P) in bf16: 12/16 of the keys in
            # fp8 cuts the fp8 quantization error by sqrt(4/3) for margin
            # against the 2e-2 gate
            v16_sb = v_pool.tile([128, 4, HL, 65], bf16, tag="v16",
                                 name="v16")
            ot_sb = [ot_pool.tile([128, seq], bf16, tag=f"ot{p}",
                                  name=f"ot{p}") for p in range(PAIRS)]

            # ---- input DMAs: big-row weight blobs first, then xT in
            # two half-tiles per k-tile (2KB rows keep DMA efficient; the
            # first half unlocks the prologue). ----
            nc.sync.dma_start(out=wqk_sb, in_=wqk_d)
            nc.sync.dma_start(out=wv2_sb, in_=wv2_d)
            HS = seq // 2
            for half in range(2):
                for k in range(KT):
                    nc.sync.dma_start(
                        out=xt_sb[k][:, half * HS:(half + 1) * HS],
                        in_=xT_d[k * 128:(k + 1) * 128, half * HS:(half + 1) * HS])
                if half == 0:
                    nc.sync.dma_start(out=bqk_sb, in_=bqk_d)
                    nc.sync.dma_start(out=bv_sb, in_=bv_d)
            for p in range(PAIRS):
                nc.sync.dma_start(out=wo_sb[p],
                                  in_=wo_d[p * 128:(p + 1) * 128, :])
            nc.vector.memset(ones_sb, 1.0)
            ones32_sb = w_pool.tile([1, 64], f32, tag="ones32", name="ones32")
            nc.vector.memset(ones32_sb, 1.0)
            # Zero-fill v_sb (pad cols could decode as fp8 NaN), then the
            # fused-denominator ones column.
            nc.vector.memset(v_sb, 0.0)
            nc.vector.memset(v_sb[:, :, :, :, 64:65], 1.0)
            nc.vector.memset(v16_sb, 0.0)
            nc.vector.memset(v16_sb[:, :, :, 64:65], 1.0)
            # Warm the exp table set during the DMA ramp.
            warm = w_pool.tile([1, 8], f32, tag="warm", name="warm")
            nc.vector.memset(warm, 0.0)
            nc.scalar.activation(out=warm, in_=warm, func=AF.Exp)
            # Keep the PE active until the first xT tiles land so the HAM
            # clock gate is already 8/8 (2.4GHz) for the prologue.
            wps = mm_pool.tile([128, 512], f32, tag="mm", name="warmps")
            for i in range(80):
                nc.tensor.matmul(wps[:, 0:128], lhsT=ones_sb[0:1, :128],
                                 rhs=ones_sb[0:1, :128],
                                 start=(i == 0), stop=(i == 79))
            warm2 = w_pool.tile([1, 8], f32, tag="warm2", name="warm2")
            nc.vector.tensor_copy(warm2, wps[0:1, 0:8])

            # ---- emission helpers ----
            # Tile only tracks RAW deps for writers EMITTED before readers,
            # so all gating below is about emission order: readers pump the
            # pending queue until their writers have been emitted.
            q_done = {0: 0, 1: 0}   # qt chunks emitted per pair
            k_done = {0: 0, 1: 0}   # kt chunks emitted per pair
            v_done = [0]            # v block-pairs emitted

            def qk_bias_store(which, p, nck, ps):
                """psum -> qt/kt with per-partition bias add on DVE."""
                dst = qt_sb[p] if which == 0 else kt_sb[p]
                col = which * 2 + p
                nc.vector.tensor_scalar_add(
                    dst[:, nck * NCH:(nck + 1) * NCH], ps[:, :NCH],
                    bqk_sb[:, col:col + 1])
                if which == 0:
                    q_done[p] = max(q_done[p], nck + 1)
                else:
                    k_done[p] = max(k_done[p], nck + 1)

            def store_v_pair(g, ps):
                """V blocks 2g,2g+1 psum -> v_sb (fp8 via ACT copy; only
                ScalarE can convert to fp8) or v16_sb (bf16 via DVE)."""
                if g >= 6:
                    nc.vector.tensor_copy(
                        v16_sb[:, (g - 6) * 2:(g - 6) * 2 + 2, :, 0:DH],
                        ps.rearrange("p (s h d) -> p s h d", s=2, h=HL))
                else:
                    nc.scalar.activation(
                        out=v_sb[:, g, :, :, 0:DH],
                        in_=ps.rearrange("p (s h d) -> p s h d", s=2, h=HL),
                        func=AF.Copy)

            def gen_v(glo, ghi):
                """V block pairs glo..ghi-1: [128, 2*J] + bias, fp8 store.

                Each pair's bias matmul + ScalarE fp8 copy is deferred until
                after the NEXT pair's matmuls, so the ACT FIFO never parks
                on a V psum that the PE hasn't finished accumulating."""
                prev = None
                for g in range(glo, ghi):
                    ps = mm_pool.tile([128, 512], f32, tag="mm", name=f"psv{g}")
                    for k in range(KT):
                        for s in range(2):
                            # start=True zeroes the touched partitions across
                            # the FULL bank width, so only the very first
                            # matmul of the tile may carry it; the second
                            # sub-chain accumulates onto pending-zero.
                            nc.tensor.matmul(
                                ps[:, s * J:(s + 1) * J],
                                lhsT=xt_sb[k][:, (2 * g + s) * 128:
                                              (2 * g + s + 1) * 128],
                                rhs=wv_sb[k],
                                start=(k == 0 and s == 0), stop=False)
                        yield
                        if k == 3 and prev is not None:
                            pg, pps = prev
                            nc.tensor.matmul(pps, lhsT=ones_sb[0:1, :128],
                                             rhs=bv_sb, start=False, stop=True)
                            store_v_pair(pg, pps)
                            v_done[0] = max(v_done[0], pg + 1)
                            prev = None
                            yield
                    prev = (g, ps)
                pg, pps = prev
                nc.tensor.matmul(pps, lhsT=ones_sb[0:1, :128],
                                 rhs=bv_sb, start=False, stop=True)
                store_v_pair(pg, pps)
                v_done[0] = max(v_done[0], pg + 1)
                yield

            class Gen:
                """PE-work generator: .step() emits ~one matmul's worth."""
                def __init__(self, it):
                    self.it = it
                    self.done = False

                def step(self):
                    if self.done:
                        return False
                    try:
                        next(self.it)
                        return True
                    except StopIteration:
                        self.done = True
                        return False

            pending = []
            pt_dump = [None]

            def pump(n):
                while n > 0 and pending:
                    if pending[0].step():
                        n -= 1
                    else:
                        pending.pop(0)

            def drain(g):
                while g.step():
                    pass

            def normalize(p, c, h01, pso):
                """1/denominator (psum row 64) and normalize O^T.

                The approx fast-recip custom-DVE op only computes correctly
                from SBUF with base partition 0, so: copy the denominator
                row to partition 0 and recip there.  The replication across
                the 64 output partitions is a K=1 matmul (ones x recip-row)
                into a psum tile plus a DVE copy back to SBUF -- all
                on-chip, no DRAM round trips in the dependency chain."""
                dnm = sm_pool.tile([1, QC], f32, tag="dnm",
                                   name=f"dnm{p}{c}{h01}")
                nc.vector.tensor_copy(dnm, pso[DH:DH + 1, :])
                rr = sm_pool.tile([1, QC], f32, tag="rr", name=f"rr{p}{c}{h01}")
                nc.vector.reciprocal_approx_fast(out=rr, in_=dnm)
                if DEBUG_DUMP and (p, c, h01) == (0, 0, 0):
                    nc.sync.dma_start(out=dbg_dnm, in_=dnm)
                    nc.sync.dma_start(out=dbg_rr, in_=rr)
                ds = dram_pool.tile([1, QC], f32, tag="ds", name=f"ds{p}{c}{h01}")
                nc.sync.dma_start(out=ds, in_=rr)
                dsap = ds[0:1, :]
                rb = sm_pool.tile([64, QC], f32, tag="rb", name=f"rb{p}{c}{h01}")
                nc.sync.dma_start(
                    out=rb,
                    in_=bass.AP(tensor=dsap.tensor, offset=dsap.offset,
                                ap=[[0, 64], [1, QC]]))
                if h01 == 0:
                    nc.vector.tensor_mul(
                        ot_sb[p][0:64, c * QC:(c + 1) * QC], pso[0:DH, :], rb)
                else:
                    tmp = sm_pool.tile([64, QC], bf16, tag="ottmp",
                                       name=f"ottmp{p}{c}")
                    nc.vector.tensor_mul(tmp, pso[0:DH, :], rb)
                    nc.sync.dma_start(
                        out=ot_sb[p][64:128, c * QC:(c + 1) * QC], in_=tmp)

            def gen_av(p, c, pt, pt16):
                """attn @ [V|1]: fp8 DoubleRow for key pairs 0..5, plain
                bf16 matmuls for key blocks 12..15."""
                pso = [psO_pool.tile([DH + 1, QC], f32, tag="psO",
                                     name=f"psO{p}{c}{h}") for h in range(2)]
                for kbp in range(6):
                    for h01 in range(2):
                        h = p * 2 + h01
                        nc.tensor.matmul(
                            pso[h01],
                            lhsT=v_sb[:, kbp, :, h, 0:DH + 1],
                            rhs=pt[:, kbp, :, h01, :],
                            start=(kbp == 0), stop=False,
                            perf_mode=DR)
                        yield
                for kb in range(12, 16):
                    for h01 in range(2):
                        h = p * 2 + h01
                        nc.tensor.matmul(
                            pso[h01],
                            lhsT=v16_sb[:, kb - 12, h, :],
                            rhs=pt16[:, kb - 12, h01, :],
                            start=False, stop=(kb == 15))
                        yield
                for h01 in range(2):
                    normalize(p, c, h01, pso[h01])
                    yield

            def gen_y(c, on_scalar=False):
                """Output-projection partials for query blocks of chunk c.
                The final chunk's psum->sbuf copies run on ScalarE (idle
                after the last exp) to keep them off the DVE queue."""
                for qb in range(c * (QC // 128), (c + 1) * (QC // 128)):
                    yb = yb_pool.tile([128, D], bf16, tag="yb", name=f"yb{qb}")
                    for nchunk in range(D // 512):
                        ps = mm_pool.tile([128, 512], f32, tag="mm",
                                          name=f"psy{qb}{nchunk}")
                        for p in range(PAIRS):
                            nc.tensor.matmul(
                                ps,
                                lhsT=ot_sb[p][:, qb * 128:(qb + 1) * 128],
                                rhs=wo_sb[p][:, nchunk * 512:(nchunk + 1) * 512],
                                start=(p == 0), stop=(p == PAIRS - 1))
                            yield
                        if on_scalar:
                            nc.scalar.copy(
                                yb[:, nchunk * 512:(nchunk + 1) * 512], ps)
                        else:
                            nc.vector.tensor_copy(
                                yb[:, nchunk * 512:(nchunk + 1) * 512], ps)
                    nc.sync.dma_start(out=y_d[qb * 128:(qb + 1) * 128, :],
                                      in_=yb)

            # per av-matmul-step: newest exp kb it reads / V pair it reads
            AV_NEED_KB = ([2 * kbp + 1 for kbp in range(6) for _ in range(2)]
                          + [kb for kb in range(12, 16) for _ in range(2)])
            AV_NEED_VP = ([kbp for kbp in range(6) for _ in range(2)]
                          + [kb // 2 for kb in range(12, 16) for _ in range(2)])

            def gated_pump(cond, limit=4096):
                """pump pending until cond() holds (emission-order gate)."""
                while not cond() and pending and limit > 0:
                    pump(1)
                    limit -= 1
                assert cond(), "emission gate unsatisfiable: pending exhausted"

            def emit_unit(p, c, prev_av, pump_n=3, on_prev_done=None):
                """Scores + exp for one (pair, chunk) unit.

                The PREVIOUS unit's attn@V remainder (last key-block pair +
                normalize, which wait on that unit's final exps) is serviced
                in this unit's first kb slots so the next scores matmuls sit
                AHEAD of it in the PE FIFO and ScalarE never gaps at unit
                boundaries.  This unit's own attn@V runs eagerly ~4 kb
                behind its exps; `pending` fills the remaining PE slack."""
                gated_pump(lambda: q_done[p] >= c + 1)
                pt = pt_pool.tile([128, 6, 2, 2, QC], f8, tag="pt",
                                  name=f"pt{p}{c}")
                pt16 = pt_pool.tile([128, 4, 2, QC], bf16, tag="pt16",
                                    name=f"pt16{p}{c}")
                if (p, c) == (0, 0):
                    pt_dump[0] = pt
                av = Gen(gen_av(p, c, pt, pt16))
                av_steps = [0]

                def av_step():
                    if not av.done and av.step():
                        av_steps[0] += 1

                for kb in range(KB):
                    gated_pump(lambda: k_done[p] >= kb // 4 + 1)
                    ps = psS_pool.tile([128, 2, QC], f32, tag="psS",
                                       name=f"psS{p}{c}{kb}")
                    for h01 in range(2):
                        nc.tensor.matmul(
                            ps[:, h01, :],
                            lhsT=kt_sb[p][h01 * 64:(h01 + 1) * 64,
                                          kb * 128:(kb + 1) * 128],
                            rhs=qt_sb[p][h01 * 64:(h01 + 1) * 64,
                                         c * QC:(c + 1) * QC],
                            start=True, stop=True,
                            tile_position=(h01 * 64, 0))
                    nc.scalar.activation(
                        out=(pt[:, kb // 2, kb % 2, :, :] if kb < 12
                             else pt16[:, kb - 12, :, :]), in_=ps,
                        func=AF.Exp, scale=0.125)
                    if prev_av is not None and not prev_av.done:
                        prev_av.step()
                        prev_av.step()
                    if (on_prev_done is not None
                            and (prev_av is None or prev_av.done)):
                        on_prev_done()
                        on_prev_done = None
                    # eager attn@V: each matmul step may only be EMITTED
                    # once its newest exp input is (plus margin so the PE
                    # FIFO isn't parked waiting on ACT) and its V block-pair
                    # writer has been emitted.
                    for _ in range(2):
                        s = av_steps[0]
                        if (s < len(AV_NEED_KB)
                                and AV_NEED_KB[s] <= kb - 2
                                and AV_NEED_VP[s] < v_done[0]):
                            av_step()
                    pump(pump_n)
                if prev_av is not None:
                    drain(prev_av)
                if on_prev_done is not None:
                    on_prev_done()
                # remaining attn@V matmuls need all V pairs emitted first
                gated_pump(lambda: v_done[0] >= KBP)
                return av

            # ---- emission schedule ----
            # Prologue: one wave of 6 k-outer chains spread across the psum
            # pools, tracking the first xT half as it lands:
            #   K0 c0, Q0 c0, V pairs 0..3
            spread = [(mm_pool, "mm"), (psS_pool, "psS"), (psO_pool, "psO"),
                      (mm_pool, "mm"), (psS_pool, "psS"), (psO_pool, "psO")]

            def emit_wave(chains):
                tiles = {}
                for idx, ch in enumerate(chains):
                    pool, tag = spread[idx]
                    tiles[ch] = pool.tile([128, 512], f32, tag=tag,
                                          name=f"pro{ch[0]}{ch[1]}")
                for k in range(KT):
                    for kind, i in chains:
                        if kind in ("k", "q"):
                            w_t = wk_sb if kind == "k" else wq_sb
                            nc.tensor.matmul(
                                tiles[(kind, i)][:, :NCH],
                                lhsT=w_t[k][:, 0:128],
                                rhs=xt_sb[k][:, i * NCH:(i + 1) * NCH],
                                start=(k == 0), stop=(k == KT - 1))
                        else:
                            for s in range(2):
                                nc.tensor.matmul(
                                    tiles[(kind, i)][:, s * J:(s + 1) * J],
                                    lhsT=xt_sb[k][:, (2 * i + s) * 128:
                                                  (2 * i + s + 1) * 128],
                                    rhs=wv_sb[k],
                                    start=(k == 0 and s == 0), stop=False)
                for kind, i in chains:
                    if kind == "k":
                        qk_bias_store(1, 0, i, tiles[(kind, i)])
                    elif kind == "q":
                        qk_bias_store(0, 0, i, tiles[(kind, i)])
                    else:
                        nc.tensor.matmul(tiles[(kind, i)],
                                         lhsT=ones_sb[0:1, :128],
                                         rhs=bv_sb, start=False, stop=True)
                        store_v_pair(i, tiles[(kind, i)])
                        v_done[0] = max(v_done[0], i + 1)

            emit_wave([("k", 0), ("q", 0), ("v", 0)])

            # Remaining projection work, most-urgent first: K0 c1..c3 gate
            # the later scores kbs of pair-0 units; V pairs 4..7 feed the
            # eager attn@V; Q0 c1..3 gate units 1..3; pair-1 Q/K gate the
            # pair-1 units.
            def gen_qkT_chunks(which, p, cs):
                w_t = wq_sb if which == 0 else wk_sb
                for nck in cs:
                    ps = mm_pool.tile([128, 512], f32, tag="mm",
                                      name=f"psqk{which}{p}{nck}")
                    for k in range(KT):
                        nc.tensor.matmul(
                            ps[:, :NCH],
                            lhsT=w_t[k][:, p * 128:(p + 1) * 128],
                            rhs=xt_sb[k][:, nck * NCH:(nck + 1) * NCH],
                            start=(k == 0), stop=(k == KT - 1))
                        yield
                    qk_bias_store(which, p, nck, ps)
                    yield

            pending.extend([
                Gen(gen_v(1, 2)),                    # V pair 1 (xt half-0)
                Gen(gen_qkT_chunks(1, 0, [1])),      # K0 c1   (xt half-0)
                Gen(gen_qkT_chunks(0, 0, [1])),      # Q0 c1   (xt half-0)
                Gen(gen_v(2, 4)),                    # V 2,3   (xt half-0)
                Gen(gen_qkT_chunks(1, 0, [2, 3])),   # K0 c2,3 (xt half-1)
                Gen(gen_qkT_chunks(0, 0, [2, 3])),   # Q0 c2,3
                Gen(gen_v(4, KBP)),                  # V pairs 4..7
                Gen(gen_qkT_chunks(0, 1, range(NP))),  # Q1
                Gen(gen_qkT_chunks(1, 1, range(NP))),  # K1
            ])

            def delayed(gen, n):
                """Defer a generator's first emission by n pump quanta (lets
                the producer's DMA-bounce latency clear before the PE FIFO
                reaches the consumer)."""
                for _ in range(n):
                    yield
                yield from gen

            steps = [(p, c) for p in range(PAIRS) for c in range(NQ)]
            av = None
            prev_pc = None
            for (p, c) in steps:
                # as soon as the PREVIOUS unit's attn@V (incl. normalize)
                # has been fully emitted, its y chunk may be queued
                if prev_pc is not None and prev_pc[0] == 1:
                    cy = prev_pc[1]
                    hook = (lambda cc: lambda: pending.append(
                        Gen(delayed(gen_y(cc), 18))))(cy)
                else:
                    hook = None
                av = emit_unit(p, c, av, on_prev_done=hook)

                prev_pc = (p, c)
            drain(av)
            pending.append(Gen(gen_y(prev_pc[1])))
            pump(1 << 30)
            if DEBUG_DUMP:
                nc.sync.dma_start(out=dbg_qt, in_=qt_sb[0])
                nc.sync.dma_start(out=dbg_kt, in_=kt_sb[0])
                nc.sync.dma_start(
                    out=dbg_v, in_=v_sb.rearrange("p a b c d -> p (a b c d)"))
                nc.sync.dma_start(out=dbg_ot, in_=ot_sb[0])

    nc.compile()
    return nc


def _get_module(seq=S):
    if seq not in _cache:
        _cache[seq] = _build_module(seq)
    return _cache[seq]


def _make_in_maps(x, Wq, bq, Wk, bk, Wv, bv, Wo):
    import ml_dtypes
    bf16 = ml_dtypes.bfloat16
    in_maps = []
    for c in range(NCORES):
        b, hg = divmod(c, 4)
        js = slice(hg * J, (hg + 1) * J)
        # bias columns: [bq pair0, bq pair1, bk pair0, bk pair1]
        bq_s = np.asarray(bq[js], np.float32)
        bk_s = np.asarray(bk[js], np.float32)
        bqk = np.stack([bq_s[0:128], bq_s[128:256],
                        bk_s[0:128], bk_s[128:256]], axis=1).astype(np.float32)
        bv_s = np.asarray(bv[js], np.float32)
        bv2 = np.concatenate([bv_s, bv_s]).reshape(1, 2 * J)
        # weight blobs: [128, k, which, 256] -> row p = W_which[k*128+p, js]
        wqk = np.empty((128, 8, 2, 256), np.float32)
        wv2 = np.empty((128, 8, 256), np.float32)
        for kk in range(8):
            wqk[:, kk, 0, :] = np.asarray(Wq, np.float32)[kk * 128:(kk + 1) * 128, js]
            wqk[:, kk, 1, :] = np.asarray(Wk, np.float32)[kk * 128:(kk + 1) * 128, js]
            wv2[:, kk, :] = np.asarray(Wv, np.float32)[kk * 128:(kk + 1) * 128, js]
        in_maps.append({
            "xT": np.ascontiguousarray(np.asarray(x[b], np.float32).T).astype(bf16),
            "wqk": np.ascontiguousarray(wqk.reshape(128, 4096)).astype(bf16),
            "wv2": np.ascontiguousarray(wv2.reshape(128, 2048)).astype(bf16),
            "wo": np.ascontiguousarray(np.asarray(Wo, np.float32)[js, :]).astype(bf16),
            "bqk": np.ascontiguousarray(bqk),
            "bv": np.ascontiguousarray(bv2).astype(bf16),
        })
    return in_maps


def _gather(results, bo):
    y = np.zeros((B, S, D), np.float32)
    for b in range(B):
        acc = np.zeros((S, D), np.float32)
        for hg in range(4):
            acc += np.asarray(results[b * 4 + hg]["y"], np.float32)
        y[b] = acc + np.asarray(bo, np.float32)[None, :]
    return y


def run_on_hw(inputs, trace=False, **kwargs):
    """Returns (y_full, BassKernelResults)."""
    from concourse.bass_utils import run_bass_kernel_spmd
    nc = _get_module()
    in_maps = _make_in_maps(
        inputs["x"], inputs["Wq"], inputs["bq"], inputs["Wk"], inputs["bk"],
        inputs["Wv"], inputs["bv"], inputs["Wo"])
    res = run_bass_kernel_spmd(nc, in_maps, core_ids=list(range(NCORES)),
                               trace=trace, **kwargs)
    y = _gather(res.results, inputs["bo"])
    return y, res


def kernel(x, Wq, bq, Wk, bk, Wv, bv, Wo, bo):
    y, _ = run_on_hw(dict(x=x, Wq=Wq, bq=bq, Wk=Wk, bk=bk, Wv=Wv, bv=bv,
                          Wo=Wo, bo=bo))
    return y
